# revision 16
# baseline (speedup 1.0000x reference)
"""Multi-headed self-attention TRN2 kernel.

Problem: B=4, S=2048, D=1024, H=16 heads (head_dim 64), fp32.
Sharding: 8 cores = 4 batches x 2 head-groups (8 heads / 512 dims each).

Per-core plan (all matmul data in fp16: 10-bit mantissa, full PE rate,
fp32 PSUM accumulation; measured end-to-end rel err ~5e-4):
  - V projection: out[s,e] tiles, bias via K=1 ones x bias init matmul,
    evicted into V_aug layout [128, 8 heads, 65] with a ones column so
    the AV matmul (M=65) also produces softmax row-sums in partition 64.
  - Q.T/K.T projections per head-pair: out[e,s], bias added during DVE
    eviction (tensor_scalar_add with [128,1] bias AP). 1/sqrt(hd) is
    folded into Wq/bq on the host.
  - scores.T[j,i]: two K=64 matmuls row-packed at tile_position
    (0,0)/(64,0) -> one [128,1024] PSUM slab; exp without max
    subtraction (scores ~ N(0,1)) via one ACT op across both banks
    -> fp16 P.T slab.
  - AV: M=65 matmuls accumulating over j into per-chunk PSUM banks.
  - normalize: DVE evict [0:65]; rowsum row -> partition 0 via SBUF DMA;
    gpsimd partition_broadcast; DVE reciprocal_approx_fast; DVE mult.
  - pair p+1's Q/K projection matmuls are emitted interleaved with pair
    p's attention so they fill the PE bubbles left by ACT-bound exp.
Output per core: (4 pairs, 128, 2048) = O.T per pair; host reassembles.
"""
import sys

sys.path.insert(0, "/opt/trn_rl_repo")

import numpy as np
from contextlib import ExitStack

from concourse import bass, tile, bacc
from concourse.bass_utils import run_bass_kernel_spmd
import concourse.mybir as mybir

B, S, D, H = 4, 2048, 1024, 16
HD = D // H          # 64 head dim
G = 2                # head groups (tensor parallel)
EG = D // G          # 512 dims per group
NPAIR = 4            # head pairs per group
NC = 8               # cores
P = 128
NCH = S // 512       # 4 i-chunks
NJ = S // P          # 16 j-tiles
ND = D // P          # 8 d-tiles
NST = S // P         # 16 s-tiles

f32 = mybir.dt.float32
f16 = mybir.dt.float16

_CACHE = {}


def build():
    nc = bacc.Bacc("TRN2", target_bir_lowering=False, debug=False, num_devices=1)

    xT = nc.dram_tensor("xT", [D, S], f16, kind="ExternalInput").ap()
    wq4 = nc.dram_tensor("wq4", [NPAIR, P, ND, P], f16, kind="ExternalInput").ap()
    wk4 = nc.dram_tensor("wk4", [NPAIR, P, ND, P], f16, kind="ExternalInput").ap()
    wv3 = nc.dram_tensor("wv3", [P, ND, EG], f16, kind="ExternalInput").ap()
    bq2 = nc.dram_tensor("bq2", [P, NPAIR], f32, kind="ExternalInput").ap()
    bk2 = nc.dram_tensor("bk2", [P, NPAIR], f32, kind="ExternalInput").ap()
    bv2 = nc.dram_tensor("bv2", [1, EG], f32, kind="ExternalInput").ap()
    out = nc.dram_tensor("out", [NPAIR, P, S], f32, kind="ExternalOutput").ap()

    with tile.TileContext(nc) as tc, ExitStack() as ctx:
        cpool = ctx.enter_context(tc.tile_pool(name="const", bufs=1))
        xpool = ctx.enter_context(tc.tile_pool(name="x", bufs=1))
        vpool = ctx.enter_context(tc.tile_pool(name="vaug", bufs=1))
        qkpool = ctx.enter_context(tc.tile_pool(name="qk", bufs=1))
        wvpool = ctx.enter_context(tc.tile_pool(name="wv", bufs=1))
        wpool = ctx.enter_context(tc.tile_pool(name="w", bufs=2))
        ptpool = ctx.enter_context(tc.tile_pool(name="pt", bufs=3))
        stpool = ctx.enter_context(tc.tile_pool(name="st", bufs=4))
        rspool = ctx.enter_context(tc.tile_pool(name="rs", bufs=4))
        npool = ctx.enter_context(tc.tile_pool(name="nrm", bufs=4))
        opool = ctx.enter_context(tc.tile_pool(name="o", bufs=4))
        # PSUM: proj 2x1 + scores 2x2 + O.T 2x1 banks = 8
        pjpool = ctx.enter_context(tc.tile_pool(name="pj", bufs=2, space="PSUM"))
        scpool = ctx.enter_context(tc.tile_pool(name="sc", bufs=2, space="PSUM"))
        popool = ctx.enter_context(tc.tile_pool(name="po", bufs=2, space="PSUM"))

        # ---- constant/bias/weight loads (weights before x: critical path)
        bqt = cpool.tile([P, NPAIR], f32)
        bkt = cpool.tile([P, NPAIR], f32)
        bvt = cpool.tile([1, EG], f32)
        nc.sync.dma_start(bqt[:], bq2)
        nc.sync.dma_start(bkt[:], bk2)
        nc.sync.dma_start(bvt[:], bv2)

        vaug = [vpool.tile([P, 8, 65], f16, name=f"vaug{i}") for i in range(NST)]
        qts = [qkpool.tile([P, S], f16, name=f"qt{p}") for p in range(NPAIR)]
        kts = [qkpool.tile([P, S], f16, name=f"kt{p}") for p in range(NPAIR)]
        wqs, wks = {}, {}

        def load_w(p):
            wq = wpool.tile([P, ND, P], f16, name="wq")
            wk = wpool.tile([P, ND, P], f16, name="wk")
            nc.sync.dma_start(wq[:], wq4[p])
            nc.sync.dma_start(wk[:], wk4[p])
            wqs[p], wks[p] = wq, wk

        load_w(0)
        wv = wvpool.tile([P, ND, EG], f16)
        nc.sync.dma_start(wv[:], wv3)
        xt = []
        for t in range(ND):
            xtile = xpool.tile([P, S], f16, name=f"xt{t}")
            nc.sync.dma_start(xtile[:], xT[t * P:(t + 1) * P, :])
            xt.append(xtile)

        ones_col = cpool.tile([1, P], f16)
        nc.vector.memset(ones_col[:], 1.0)
        bvt16 = cpool.tile([1, EG], f16)
        nc.vector.tensor_copy(bvt16[:], bvt[:])

        def proj_chunk(p, c, which):
            """Generator: one 512-col chunk of the Q.T (which=0) / K.T
            (which=1) projection for pair p. Yields after each engine op
            so two chains can be zipped (alternating PSUM banks lets the
            PE overlap fill/drain across chains)."""
            cs = slice(c * 512, (c + 1) * 512)
            w = wqs[p] if which == 0 else wks[p]
            dst = qts[p] if which == 0 else kts[p]
            bias = bqt if which == 0 else bkt
            pp = pjpool.tile([P, 512], f32, name="pj")
            for t in range(ND):
                nc.tensor.matmul(pp[:], w[:, t, :], xt[t][:, cs],
                                 start=(t == 0), stop=(t == ND - 1))
                yield
            nc.vector.tensor_scalar_add(dst[:, cs], pp[:], bias[:, p:p + 1])
            yield

        def v_proj(st_i, half):
            """V projection for s-tile st_i (full width; half ignored -> only
            emit on half==0)."""
            if half == 1:
                return
            pv = pjpool.tile([P, 512], f32, name="pj")
            nc.tensor.matmul(pv[:], ones_col[:], bvt16[:], start=True, stop=False)
            yield
            for t in range(ND):
                nc.tensor.matmul(pv[:], xt[t][:, st_i * P:(st_i + 1) * P],
                                 wv[:, t, :], start=False, stop=(t == ND - 1))
                yield
            nc.vector.memset(vaug[st_i][:, :, 64:65], 1.0)
            nc.vector.tensor_copy(vaug[st_i][:, :, 0:64],
                                  pv[:].rearrange("p (h e) -> p h e", h=8))
            yield

        # ---- pending projection chains, drip-fed into attention bubbles
        from collections import deque

        class Chain:
            __slots__ = ("gen", "done")

            def __init__(self, gen):
                self.gen = gen
                self.done = False

            def adv(self):
                if not self.done:
                    try:
                        next(self.gen)
                    except StopIteration:
                        self.done = True
                return not self.done

        pending = deque()
        active = []

        def step(n):
            """Advance up to n engine-ops, zipping at most two chains at a
            time (alternating PSUM banks) in FIFO order."""
            emitted = 0
            while emitted < n:
                while len(active) < 2 and pending:
                    active.append(pending.popleft())
                if not active:
                    return
                ch = active.pop(0)
                if ch.adv():
                    active.append(ch)
                    emitted += 1

        def require(ch):
            """Emit everything up to and including chain ch."""
            if ch is None or ch.done:
                return
            while not ch.done:
                step(8)

        def drain():
            while pending or active:
                step(64)

        vmap = {}     # (s_tile, half) -> Chain
        qmap = {}     # (pair, chunk) -> Chain  (Q side; K handled at pair)
        kchains = {}  # pair -> list of Chains

        def enq(ch, front=False):
            ch = Chain(ch)
            (active.insert(0, ch) if front else pending.append(ch))
            return ch

        # ---- critical path to first exp: K0 (all chunks), Q0 chunk 0,
        #      V half-A (pairs 0/1) s0..s7.  The rest threads in later.
        def run(*chains):
            live = [Chain(g) for g in chains]
            while live:
                live = [ch for ch in live if ch.adv()]

        run(proj_chunk(0, 0, 1), proj_chunk(0, 1, 1))
        run(proj_chunk(0, 2, 1), proj_chunk(0, 3, 1))
        run(proj_chunk(0, 0, 0), v_proj(0, 0), v_proj(1, 0))
        run(v_proj(2, 0), v_proj(3, 0), v_proj(4, 0))
        run(v_proj(5, 0), v_proj(6, 0), v_proj(7, 0))
        for s in range(8):
            vmap[(s, 0)] = Chain(iter(()))
            vmap[(s, 0)].done = True

        # ---- attention per pair; projections threaded into ACT bubbles
        for p in range(NPAIR):
            qt, kt = qts[p], kts[p]
            if p == 0:
                for s in range(8, NST):
                    vmap[(s, 0)] = enq(v_proj(s, 0))    # vaug half-A tail
                for s in range(NST):
                    vmap[(s, 1)] = enq(v_proj(s, 1))    # vaug half-B
            if p + 1 < NPAIR:
                load_w(p + 1)
                kchains[p + 1] = [enq(proj_chunk(p + 1, c, 1))
                                  for c in range(NCH)]
                qmap[(p + 1, 0)] = enq(proj_chunk(p + 1, 0, 0))
            half = p // 2
            if p >= 1:
                for ch in kchains.get(p, []):
                    require(ch)
            for c in range(NCH):
                cs = slice(c * 512, (c + 1) * 512)
                if p == 0 and c + 1 < NCH:
                    qmap[(0, c + 1)] = enq(proj_chunk(0, c + 1, 0), front=True)
                if p + 1 < NPAIR and c + 1 < NCH:
                    qmap[(p + 1, c + 1)] = enq(proj_chunk(p + 1, c + 1, 0))
                require(qmap.get((p, c)))
                otA = popool.tile([P, 512], f32, name="po")
                otB = popool.tile([P, 512], f32, name="po")
                for j in range(NJ):
                    js = slice(j * P, (j + 1) * P)
                    ps = scpool.tile([P, 1024], f32, name="sc")
                    nc.tensor.matmul(ps[:, 0:512], kt[0:64, js],
                                     qt[0:64, cs], start=True, stop=True,
                                     tile_position=(0, 0))
                    nc.tensor.matmul(ps[:, 512:1024], kt[64:128, js],
                                     qt[64:128, cs], start=True, stop=True,
                                     tile_position=(64, 0))
                    pt = ptpool.tile([P, 1024], f16, name="pt")
                    nc.scalar.activation(pt[:], ps[:],
                                         mybir.ActivationFunctionType.Exp)
                    step(6 if (p == 0 and c == 0) else 4)
                    require(vmap[(j, half)])
                    nc.tensor.matmul(otA[0:65, :],
                                     vaug[j][:, 2 * p:2 * p + 1, :],
                                     pt[:, 0:512],
                                     start=(j == 0), stop=(j == NJ - 1))
                    nc.tensor.matmul(otB[0:65, :],
                                     vaug[j][:, 2 * p + 1:2 * p + 2, :],
                                     pt[:, 512:1024],
                                     start=(j == 0), stop=(j == NJ - 1))
                # per-chunk evict + normalize + emit (keeps the tail short)
                for h, ot in ((0, otA), (1, otB)):
                    sth = stpool.tile([P, 512], f32, name="st")
                    nc.vector.tensor_copy(sth[0:65, :], ot[0:65, :])
                    rs0 = rspool.tile([1, 512], f32, name="rs0")
                    nc.sync.dma_start(rs0[:], sth[64:65, :])
                    bc = npool.tile([64, 512], f32, name="bc")
                    nc.gpsimd.partition_broadcast(bc[:], rs0[:])
                    rbc = npool.tile([64, 512], f32, name="rbc")
                    nc.vector.reciprocal_approx_fast(out=rbc[:], in_=bc[:])
                    no = opool.tile([64, 512], f32, name="no")
                    nc.vector.tensor_mul(no[:], sth[0:64, :], rbc[:])
                    nc.sync.dma_start(out[p, h * 64:(h + 1) * 64, cs], no[:])
        drain()

    nc.compile()
    return nc


def _host_prep(x, Wq, bq, Wk, bk, Wv, bv):
    """Build the 8 per-core input maps."""
    x = np.asarray(x, dtype=np.float32)
    scale = np.float32(1.0 / np.sqrt(HD))
    in_maps = []
    for core in range(NC):
        b, g = divmod(core, G)
        rows = slice(g * EG, (g + 1) * EG)
        wq_g = np.asarray(Wq, np.float32)[rows] * scale
        bq_g = np.asarray(bq, np.float32)[rows] * scale
        wk_g = np.asarray(Wk, np.float32)[rows]
        bk_g = np.asarray(bk, np.float32)[rows]
        wv_g = np.asarray(Wv, np.float32)[rows]
        bv_g = np.asarray(bv, np.float32)[rows]
        # wq4[p, dd, t, c] = wq_g.T[t*128+dd, p*128+c]
        wq4 = np.ascontiguousarray(
            wq_g.T.reshape(ND, P, NPAIR, P).transpose(2, 1, 0, 3)
        ).astype(np.float16)
        wk4 = np.ascontiguousarray(
            wk_g.T.reshape(ND, P, NPAIR, P).transpose(2, 1, 0, 3)
        ).astype(np.float16)
        wv3 = np.ascontiguousarray(
            wv_g.T.reshape(ND, P, EG).transpose(1, 0, 2)
        ).astype(np.float16)
        in_maps.append({
            "xT": np.ascontiguousarray(x[b].T).astype(np.float16),
            "wq4": wq4,
            "wk4": wk4,
            "wv3": wv3,
            "bq2": np.ascontiguousarray(bq_g.reshape(NPAIR, P).T),
            "bk2": np.ascontiguousarray(bk_g.reshape(NPAIR, P).T),
            "bv2": np.ascontiguousarray(bv_g.reshape(1, EG)),
        })
    return in_maps


def _assemble(results):
    """results: list of 8 dicts with 'out' (4, 128, 2048) -> (B, S, D)."""
    full = np.empty((B, S, D), dtype=np.float32)
    for core in range(NC):
        b, g = divmod(core, G)
        co = results[core]["out"]          # (NPAIR, P, S)
        full[b, :, g * EG:(g + 1) * EG] = (
            co.transpose(2, 0, 1).reshape(S, EG))
    return full


def _get_nc():
    if "nc" not in _CACHE:
        _CACHE["nc"] = build()
    return _CACHE["nc"]


def kernel(x, Wq, bq, Wk, bk, Wv, bv):
    nc = _get_nc()
    in_maps = _host_prep(x, Wq, bq, Wk, bk, Wv, bv)
    res = run_bass_kernel_spmd(nc, in_maps, list(range(NC)))
    return _assemble(res.results)


# revision 17
# speedup vs baseline: 1.0184x; 1.0184x over previous
"""Multi-headed self-attention TRN2 kernel.

Problem: B=4, S=2048, D=1024, H=16 heads (head_dim 64), fp32.
Sharding: 8 cores = 4 batches x 2 head-groups (8 heads / 512 dims each).

Per-core plan (all matmul data in fp16: 10-bit mantissa, full PE rate,
fp32 PSUM accumulation; measured end-to-end rel err ~5e-4):
  - V projection: out[s,e] tiles, bias via K=1 ones x bias init matmul,
    evicted into V_aug layout [128, 8 heads, 65] with a ones column so
    the AV matmul (M=65) also produces softmax row-sums in partition 64.
  - Q.T/K.T projections per head-pair: out[e,s], bias added during DVE
    eviction (tensor_scalar_add with [128,1] bias AP). 1/sqrt(hd) is
    folded into Wq/bq on the host.
  - scores.T[j,i]: two K=64 matmuls row-packed at tile_position
    (0,0)/(64,0) -> one [128,1024] PSUM slab; exp without max
    subtraction (scores ~ N(0,1)) via one ACT op across both banks
    -> fp16 P.T slab.
  - AV: M=65 matmuls accumulating over j into per-chunk PSUM banks.
  - normalize: DVE evict [0:65]; rowsum row -> partition 0 via SBUF DMA;
    gpsimd partition_broadcast; DVE reciprocal_approx_fast; DVE mult.
  - pair p+1's Q/K projection matmuls are emitted interleaved with pair
    p's attention so they fill the PE bubbles left by ACT-bound exp.
Output per core: (4 pairs, 128, 2048) = O.T per pair; host reassembles.
"""
import sys

sys.path.insert(0, "/opt/trn_rl_repo")

import numpy as np
from contextlib import ExitStack

from concourse import bass, tile, bacc
from concourse.bass_utils import run_bass_kernel_spmd
import concourse.mybir as mybir

B, S, D, H = 4, 2048, 1024, 16
HD = D // H          # 64 head dim
G = 2                # head groups (tensor parallel)
EG = D // G          # 512 dims per group
NPAIR = 4            # head pairs per group
NC = 8               # cores
P = 128
NCH = S // 512       # 4 i-chunks
NJ = S // P          # 16 j-tiles
ND = D // P          # 8 d-tiles
NST = S // P         # 16 s-tiles

f32 = mybir.dt.float32
f16 = mybir.dt.float16

_CACHE = {}


def build():
    nc = bacc.Bacc("TRN2", target_bir_lowering=False, debug=False, num_devices=1)

    xT = nc.dram_tensor("xT", [D, S], f16, kind="ExternalInput").ap()
    wq4 = nc.dram_tensor("wq4", [NPAIR, P, ND, P], f16, kind="ExternalInput").ap()
    wk4 = nc.dram_tensor("wk4", [NPAIR, P, ND, P], f16, kind="ExternalInput").ap()
    wv3 = nc.dram_tensor("wv3", [P, ND, EG], f16, kind="ExternalInput").ap()
    bq2 = nc.dram_tensor("bq2", [P, NPAIR], f32, kind="ExternalInput").ap()
    bk2 = nc.dram_tensor("bk2", [P, NPAIR], f32, kind="ExternalInput").ap()
    bv2 = nc.dram_tensor("bv2", [1, EG], f32, kind="ExternalInput").ap()
    out = nc.dram_tensor("out", [NPAIR, P, S], f32, kind="ExternalOutput").ap()

    with tile.TileContext(nc) as tc, ExitStack() as ctx:
        cpool = ctx.enter_context(tc.tile_pool(name="const", bufs=1))
        xpool = ctx.enter_context(tc.tile_pool(name="x", bufs=1))
        vpool = ctx.enter_context(tc.tile_pool(name="vaug", bufs=1))
        qkpool = ctx.enter_context(tc.tile_pool(name="qk", bufs=1))
        wvpool = ctx.enter_context(tc.tile_pool(name="wv", bufs=1))
        wpool = ctx.enter_context(tc.tile_pool(name="w", bufs=2))
        ptpool = ctx.enter_context(tc.tile_pool(name="pt", bufs=3))
        stpool = ctx.enter_context(tc.tile_pool(name="st", bufs=4))
        rspool = ctx.enter_context(tc.tile_pool(name="rs", bufs=4))
        npool = ctx.enter_context(tc.tile_pool(name="nrm", bufs=4))
        opool = ctx.enter_context(tc.tile_pool(name="o", bufs=4))
        # PSUM: proj 2x1 + scores 2x2 + O.T 2x1 banks = 8
        pjpool = ctx.enter_context(tc.tile_pool(name="pj", bufs=2, space="PSUM"))
        scpool = ctx.enter_context(tc.tile_pool(name="sc", bufs=2, space="PSUM"))
        popool = ctx.enter_context(tc.tile_pool(name="po", bufs=2, space="PSUM"))

        # ---- constant/bias/weight loads (weights before x: critical path)
        bqt = cpool.tile([P, NPAIR], f32)
        bkt = cpool.tile([P, NPAIR], f32)
        bvt = cpool.tile([1, EG], f32)
        nc.sync.dma_start(bqt[:], bq2)
        nc.sync.dma_start(bkt[:], bk2)
        nc.sync.dma_start(bvt[:], bv2)

        vaug = [vpool.tile([P, 8, 65], f16, name=f"vaug{i}") for i in range(NST)]
        qts = [qkpool.tile([P, S], f16, name=f"qt{p}") for p in range(NPAIR)]
        kts = [qkpool.tile([P, S], f16, name=f"kt{p}") for p in range(NPAIR)]
        wqs, wks = {}, {}

        def load_w(p):
            wq = wpool.tile([P, ND, P], f16, name="wq")
            wk = wpool.tile([P, ND, P], f16, name="wk")
            nc.sync.dma_start(wq[:], wq4[p])
            nc.sync.dma_start(wk[:], wk4[p])
            wqs[p], wks[p] = wq, wk

        load_w(0)
        wv = wvpool.tile([P, ND, EG], f16)
        nc.sync.dma_start(wv[:], wv3)
        xt = []
        for t in range(ND):
            xtile = xpool.tile([P, S], f16, name=f"xt{t}")
            nc.sync.dma_start(xtile[:], xT[t * P:(t + 1) * P, :])
            xt.append(xtile)

        # broadcast bv across partitions once (for the DVE bias add)
        bvbc = cpool.tile([P, EG], f32)
        nc.gpsimd.partition_broadcast(bvbc[:], bvt[:])

        def proj_chunk(p, c, which):
            """Generator: one 512-col chunk of the Q.T (which=0) / K.T
            (which=1) projection for pair p. Yields after each engine op
            so two chains can be zipped (alternating PSUM banks lets the
            PE overlap fill/drain across chains)."""
            cs = slice(c * 512, (c + 1) * 512)
            w = wqs[p] if which == 0 else wks[p]
            dst = qts[p] if which == 0 else kts[p]
            bias = bqt if which == 0 else bkt
            pp = pjpool.tile([P, 512], f32, name="pj")
            for t in range(ND):
                nc.tensor.matmul(pp[:], w[:, t, :], xt[t][:, cs],
                                 start=(t == 0), stop=(t == ND - 1))
                yield
            nc.vector.tensor_scalar_add(dst[:, cs], pp[:], bias[:, p:p + 1])
            yield

        def v_proj(st_i, half):
            """V projection for s-tile st_i, heads [4*half, 4*half+4)."""
            es = slice(half * 256, (half + 1) * 256)
            pv = pjpool.tile([P, 512], f32, name="pj")
            for t in range(ND):
                nc.tensor.matmul(pv[:, 0:256], xt[t][:, st_i * P:(st_i + 1) * P],
                                 wv[:, t, es], start=(t == 0), stop=(t == ND - 1))
                yield
            hs = slice(4 * half, 4 * half + 4)
            nc.vector.memset(vaug[st_i][:, hs, 64:65], 1.0)
            nc.vector.tensor_tensor(
                vaug[st_i][:, hs, 0:64],
                pv[:, 0:256].rearrange("p (h e) -> p h e", h=4),
                bvbc[:, es].rearrange("p (h e) -> p h e", h=4),
                op=mybir.AluOpType.add)
            yield

        # ---- pending projection chains, drip-fed into attention bubbles
        from collections import deque

        class Chain:
            __slots__ = ("gen", "done")

            def __init__(self, gen):
                self.gen = gen
                self.done = False

            def adv(self):
                if not self.done:
                    try:
                        next(self.gen)
                    except StopIteration:
                        self.done = True
                return not self.done

        pending = deque()
        active = []

        def step(n):
            """Advance up to n engine-ops, zipping at most two chains at a
            time (alternating PSUM banks) in FIFO order."""
            emitted = 0
            while emitted < n:
                while len(active) < 2 and pending:
                    active.append(pending.popleft())
                if not active:
                    return
                ch = active.pop(0)
                if ch.adv():
                    active.append(ch)
                    emitted += 1

        def require(ch):
            """Emit everything up to and including chain ch."""
            if ch is None or ch.done:
                return
            while not ch.done:
                step(8)

        def drain():
            while pending or active:
                step(64)

        vmap = {}     # (s_tile, half) -> Chain
        qmap = {}     # (pair, chunk) -> Chain  (Q side; K handled at pair)
        kchains = {}  # pair -> list of Chains

        def enq(ch, front=False):
            ch = Chain(ch)
            (active.insert(0, ch) if front else pending.append(ch))
            return ch

        # ---- critical path to first exp: K0 (all chunks), Q0 chunk 0,
        #      V half-A (pairs 0/1) s0..s7.  The rest threads in later.
        def run(*chains):
            live = [Chain(g) for g in chains]
            while live:
                live = [ch for ch in live if ch.adv()]

        run(proj_chunk(0, 0, 1), proj_chunk(0, 1, 1))
        run(proj_chunk(0, 2, 1), proj_chunk(0, 3, 1))
        run(proj_chunk(0, 0, 0), v_proj(0, 0), v_proj(1, 0))
        for s in range(2):
            vmap[(s, 0)] = Chain(iter(()))
            vmap[(s, 0)].done = True

        # ---- attention per pair; projections threaded into ACT bubbles
        for p in range(NPAIR):
            qt, kt = qts[p], kts[p]
            if p == 0:
                for s in range(2, NST):
                    vmap[(s, 0)] = enq(v_proj(s, 0))    # vaug half-A tail
                for s in range(NST):
                    vmap[(s, 1)] = enq(v_proj(s, 1))    # vaug half-B
            if p + 1 < NPAIR:
                load_w(p + 1)
                kchains[p + 1] = [enq(proj_chunk(p + 1, c, 1))
                                  for c in range(NCH)]
                qmap[(p + 1, 0)] = enq(proj_chunk(p + 1, 0, 0))
            half = p // 2
            if p >= 1:
                for ch in kchains.get(p, []):
                    require(ch)
            for c in range(NCH):
                cs = slice(c * 512, (c + 1) * 512)
                if p == 0 and c + 1 < NCH:
                    qmap[(0, c + 1)] = enq(proj_chunk(0, c + 1, 0), front=True)
                if p + 1 < NPAIR and c + 1 < NCH:
                    qmap[(p + 1, c + 1)] = enq(proj_chunk(p + 1, c + 1, 0))
                require(qmap.get((p, c)))
                otA = popool.tile([P, 512], f32, name="po")
                otB = popool.tile([P, 512], f32, name="po")
                for j in range(NJ):
                    js = slice(j * P, (j + 1) * P)
                    ps = scpool.tile([P, 1024], f32, name="sc")
                    nc.tensor.matmul(ps[:, 0:512], kt[0:64, js],
                                     qt[0:64, cs], start=True, stop=True,
                                     tile_position=(0, 0))
                    nc.tensor.matmul(ps[:, 512:1024], kt[64:128, js],
                                     qt[64:128, cs], start=True, stop=True,
                                     tile_position=(64, 0))
                    pt = ptpool.tile([P, 1024], f16, name="pt")
                    nc.scalar.activation(pt[:], ps[:],
                                         mybir.ActivationFunctionType.Exp)
                    step(6 if (p == 0 and c == 0) else 4)
                    require(vmap[(j, half)])
                    nc.tensor.matmul(otA[0:65, :],
                                     vaug[j][:, 2 * p:2 * p + 1, :],
                                     pt[:, 0:512],
                                     start=(j == 0), stop=(j == NJ - 1))
                    nc.tensor.matmul(otB[0:65, :],
                                     vaug[j][:, 2 * p + 1:2 * p + 2, :],
                                     pt[:, 512:1024],
                                     start=(j == 0), stop=(j == NJ - 1))
                # per-chunk evict + normalize + emit (keeps the tail short)
                for h, ot in ((0, otA), (1, otB)):
                    sth = stpool.tile([P, 512], f32, name="st")
                    nc.vector.tensor_copy(sth[0:65, :], ot[0:65, :])
                    rs0 = rspool.tile([1, 512], f32, name="rs0")
                    nc.sync.dma_start(rs0[:], sth[64:65, :])
                    bc = npool.tile([64, 512], f32, name="bc")
                    nc.gpsimd.partition_broadcast(bc[:], rs0[:])
                    rbc = npool.tile([64, 512], f32, name="rbc")
                    nc.vector.reciprocal_approx_fast(out=rbc[:], in_=bc[:])
                    no = opool.tile([64, 512], f32, name="no")
                    nc.vector.tensor_mul(no[:], sth[0:64, :], rbc[:])
                    nc.sync.dma_start(out[p, h * 64:(h + 1) * 64, cs], no[:])
        drain()

    nc.compile()
    return nc


def _host_prep(x, Wq, bq, Wk, bk, Wv, bv):
    """Build the 8 per-core input maps."""
    x = np.asarray(x, dtype=np.float32)
    scale = np.float32(1.0 / np.sqrt(HD))
    in_maps = []
    for core in range(NC):
        b, g = divmod(core, G)
        rows = slice(g * EG, (g + 1) * EG)
        wq_g = np.asarray(Wq, np.float32)[rows] * scale
        bq_g = np.asarray(bq, np.float32)[rows] * scale
        wk_g = np.asarray(Wk, np.float32)[rows]
        bk_g = np.asarray(bk, np.float32)[rows]
        wv_g = np.asarray(Wv, np.float32)[rows]
        bv_g = np.asarray(bv, np.float32)[rows]
        # wq4[p, dd, t, c] = wq_g.T[t*128+dd, p*128+c]
        wq4 = np.ascontiguousarray(
            wq_g.T.reshape(ND, P, NPAIR, P).transpose(2, 1, 0, 3)
        ).astype(np.float16)
        wk4 = np.ascontiguousarray(
            wk_g.T.reshape(ND, P, NPAIR, P).transpose(2, 1, 0, 3)
        ).astype(np.float16)
        wv3 = np.ascontiguousarray(
            wv_g.T.reshape(ND, P, EG).transpose(1, 0, 2)
        ).astype(np.float16)
        in_maps.append({
            "xT": np.ascontiguousarray(x[b].T).astype(np.float16),
            "wq4": wq4,
            "wk4": wk4,
            "wv3": wv3,
            "bq2": np.ascontiguousarray(bq_g.reshape(NPAIR, P).T),
            "bk2": np.ascontiguousarray(bk_g.reshape(NPAIR, P).T),
            "bv2": np.ascontiguousarray(bv_g.reshape(1, EG)),
        })
    return in_maps


def _assemble(results):
    """results: list of 8 dicts with 'out' (4, 128, 2048) -> (B, S, D)."""
    full = np.empty((B, S, D), dtype=np.float32)
    for core in range(NC):
        b, g = divmod(core, G)
        co = results[core]["out"]          # (NPAIR, P, S)
        full[b, :, g * EG:(g + 1) * EG] = (
            co.transpose(2, 0, 1).reshape(S, EG))
    return full


def _get_nc():
    if "nc" not in _CACHE:
        _CACHE["nc"] = build()
    return _CACHE["nc"]


def kernel(x, Wq, bq, Wk, bk, Wv, bv):
    nc = _get_nc()
    in_maps = _host_prep(x, Wq, bq, Wk, bk, Wv, bv)
    res = run_bass_kernel_spmd(nc, in_maps, list(range(NC)))
    return _assemble(res.results)
